# revision 37
# baseline (speedup 1.0000x reference)
"""Causal self-attention (B=2, L=2048, D=1024, H=16) on 8 trn2 NeuronCores.

Sharding: core c = 4*b + g handles batch b and head group g (4 heads).
Per core: QKV projection for its heads' weight columns (tensor-parallel),
flash-style causal attention for its 4 heads, and a partial output
projection over its 256 head-dims (row-parallel).  The host sums the 4
partial projections per batch and adds bproj.

v2 design (single interleaved instruction stream, all-bf16 matmuls):
  - All matmul operands bf16 (FWL weight loads, half DMA); psum stays f32.
  - One software-pipelined stream: QKV chunk s+1 and output-projection
    chunk s-1 matmuls are pulled as PE "filler" between attention steps,
    paced by an emitted-time estimator, so the PE never idles while the
    scalar engine grinds through exps (ACT is the attention-phase
    co-bottleneck at ~1 elem/lane/cycle + ~350cy/call overhead).
  - Score psums are [128, 1024] (2 banks): both heads of a pair per
    k-tile, one merged exp call per step (halves ACT call overhead).
  - Causal diagonal masked ON THE PE: an extra accumulation matmul
    negi^T @ u128 adds -1e30 above the diagonal (no DVE mask adds).
  - AV rides the ones-column trick for Z (col 64 of each head group).
  - Normalize: 1/Z via single-pass custom-DVE reciprocal_approx_fast
    (~51 ULP) into an f32r tile, broadcast across partitions with a
    K=1 ones matmul, psum->sbuf copies, then in-partition multiplies
    reading the AV psum directly (no separate un-normalized copy);
    head1's half goes through a tmp tile + SBUF->SBUF DMA since matmul
    psum bases are restricted to partitions 0/32/64.
  - QKV/V copies out of psum moved to the vector engine (tensor_scalar
    bias-add fused); y copies stay on ACT (bias fused), output bf16.
"""

import sys
import types
from collections import deque

import numpy as np


def _install_ntff_shim():
    """The container's antenv stub lacks axon_hooks; recreate it so
    run_bass_kernel_spmd(trace=True) can reach the NTFF profiler."""
    if "antenv.axon_hooks" in sys.modules:
        return
    try:
        import antenv
        from trn_agent_boot.trn_boot import _ntff_profile_via_ctypes
    except Exception:
        return
    mod = types.ModuleType("antenv.axon_hooks")
    hook = _ntff_profile_via_ctypes("/opt/axon/libaxon_pjrt.so")
    mod.get_axon_ntff_profile_hook = lambda: hook
    mod.set_axon_ntff_profile_hook = lambda h: None
    sys.modules["antenv.axon_hooks"] = mod
    antenv.axon_hooks = mod


_install_ntff_shim()

import ml_dtypes  # noqa: E402

import concourse.bass as bass  # noqa: E402
import concourse.mybir as mybir  # noqa: E402
import concourse.tile as tile  # noqa: E402
from concourse.bass_utils import run_bass_kernel_spmd  # noqa: E402
from concourse.vector_clock import ScopedClock, VectorClock  # noqa: E402

B, L, D, H = 2, 2048, 1024, 16
HD = D // H  # 64
N_CORES = 8
HPC = 4  # heads per core
CD = HPC * HD  # 256 head-dims per core
VW = HPC * (HD + 1)  # 260 interleaved V columns
SCALE = HD**-0.5  # 0.125
F32 = mybir.dt.float32
R32 = mybir.dt.float32r
B16 = mybir.dt.bfloat16
NPB16 = ml_dtypes.bfloat16
NEG = -1.0e30

KT = L // 128  # 16 k-tiles of 128 keys
NS = L // 512  # 4 query chunks of 512
N_DK = D // 128  # 8 feature k-tiles
AV_DELAY = 4  # AV matmuls issue this many merged steps behind their exp
EXPF = mybir.ActivationFunctionType.Exp
IDF = mybir.ActivationFunctionType.Identity
LNF = mybir.ActivationFunctionType.Ln
MULT = mybir.AluOpType.mult


class _TileContext(tile.TileContext):
    """Split exit-drain sem waits to 1 per drain; this walrus build's
    CTRL codegen rejects drains with 2+ sync waits."""

    def _drain_and_barrier(self, tick_clock, wait_clock):
        g = tick_clock.global_clock
        n = len(g)
        procs = [i for i in range(n) if g[i] > 0]
        for p in procs:
            vec = [g[i] if i == p else 0 for i in range(n)]
            d = self.nc.sync.drain()
            wait_clock.add_sem_waits(d.ins, ScopedClock({None: VectorClock(vec)}))
        self.nc.all_engine_barrier()
        popped = self.nc._tile_sem_poison_stack.pop()
        assert popped is self._sem_poison
        self.nc.clear_and_free_semaphores(list(self.sems.allocated().values()))
        self.nc.all_engine_barrier()


def _split_multi_waits(nc):
    """This walrus build's codegen accepts only ONE sync wait per
    instruction; hoist extra waits onto preceding same-engine NOPs."""
    for f in nc.m.functions:
        for blk in f.blocks:
            orig = list(blk.instructions)
            expanded = []
            changed = False
            for ins in orig:
                si = ins.sync_info
                if si is not None and si.on_wait is not None and len(si.on_wait) > 1:
                    changed = True
                    waits = list(si.on_wait)
                    eng = nc.engines[ins.engine]
                    for w in waits[:-1]:
                        nop = eng.nop(nofuse=True).ins
                        nc.cur_bb.bb.instructions.remove(nop)
                        nop.sync_info = mybir.SyncInfo(on_wait=[w], on_update=[])
                        expanded.append(nop)
                    ins.sync_info = mybir.SyncInfo(
                        on_wait=[waits[-1]], on_update=list(si.on_update or [])
                    )
                expanded.append(ins)
            if changed:
                il = blk.instructions
                for ins in list(il):
                    il.remove(ins)
                for ins in expanded:
                    il.append(ins)


def _build_program():
    nc = bass.Bass()
    xT_d = nc.dram_tensor("xT", [D, L], B16, kind="ExternalInput").ap()
    wqkv_d = nc.dram_tensor("wqkv", [D, 2 * CD + VW], B16, kind="ExternalInput").ap()
    bqk_d = nc.dram_tensor("bqk", [128, 4], F32, kind="ExternalInput").ap()
    bv_d = nc.dram_tensor("bv", [1, VW], B16, kind="ExternalInput").ap()
    wproj_d = nc.dram_tensor("wproj", [CD, D], B16, kind="ExternalInput").ap()
    onesb_d = nc.dram_tensor("onesb", [1, 128], B16, kind="ExternalInput").ap()
    onesr_d = nc.dram_tensor("onesr", [1, 64], R32, kind="ExternalInput").ap()
    negi_d = nc.dram_tensor("negi", [128, 128], B16, kind="ExternalInput").ap()
    u128_d = nc.dram_tensor("u128", [128, 128], B16, kind="ExternalInput").ap()
    zer_d = nc.dram_tensor("zer", [64, L], B16, kind="ExternalInput").ap()
    yT_d = nc.dram_tensor("yT", [D, L], B16, kind="ExternalOutput").ap()

    mm = nc.tensor.matmul
    E = {"pe": 0.0, "act": 0.0, "fill": 0.0, "in_fill": False}

    with _TileContext(nc) as tc, tc.tile_pool(name="sb", bufs=1) as sb, tc.tile_pool(
        name="ps", bufs=1, space="PSUM"
    ) as ps, nc.allow_low_precision(reason="bf16 attention kernel"):
        # ---- constants (issued on the ACT HWDGE ring: the sync ring's
        # trigger FIFO (~0.6us each) must stay clear for the critical
        # weight/activation loads) ----
        negi = sb.tile([128, 128], B16, tag="negi", bufs=1)
        nc.scalar.dma_start(out=negi[:], in_=negi_d[:])
        u128 = sb.tile([128, 128], B16, tag="u128", bufs=1)
        nc.scalar.dma_start(out=u128[:], in_=u128_d[:])
        onesb = sb.tile([1, 128], B16, tag="onesb", bufs=1)
        nc.scalar.dma_start(out=onesb[:], in_=onesb_d[:])
        onesr = sb.tile([1, 64], R32, tag="onesr", bufs=1)
        nc.scalar.dma_start(out=onesr[:], in_=onesr_d[:])
        bqk = sb.tile([128, 4], F32, tag="bqk", bufs=1)
        nc.scalar.dma_start(out=bqk[:], in_=bqk_d[:])
        bv = sb.tile([1, VW], B16, tag="bv", bufs=1)
        nc.scalar.dma_start(out=bv[:], in_=bv_d[:])

        # ---- persistent SBUF tensors + DMA schedule ----
        # One big tile per tensor with per-k column views: a single batched
        # DMA per group (one ~0.6us sync-ring trigger instead of 8).
        WQW = 2 * CD + VW  # 772
        wqkv_t = sb.tile([128, N_DK * WQW], B16, tag="wqkv", bufs=1, name="wqkv")
        wqkv = [wqkv_t[:, WQW * k : WQW * (k + 1)] for k in range(N_DK)]
        wqkv_src = wqkv_d.rearrange("(k p) c -> p k c", k=N_DK)
        wqkv_dst = wqkv_t[:].rearrange("p (k c) -> p k c", k=N_DK)
        xT_t = sb.tile([128, N_DK * L], B16, tag="xT", bufs=1, name="xT")
        xTc = [
            [xT_t[:, L * k + 512 * s : L * k + 512 * (s + 1)] for s in range(NS)]
            for k in range(N_DK)
        ]
        xT_src = xT_d.rearrange("(k p) c -> p k c", k=N_DK)
        xT_dst = xT_t[:].rearrange("p (k c) -> p k c", k=N_DK)
        # first-consumption order: w cols 0..255 (m0/m1), x chunk0 for k=0,
        # then the rest of x chunk0, w 256..511, V cols, zero-halves, x 1-3
        wq_groups = [(0, 256), (256, 512), (512, WQW)]
        lo_, hi_ = wq_groups[0]
        nc.sync.dma_start(out=wqkv_dst[:, :, lo_:hi_], in_=wqkv_src[:, :, lo_:hi_])
        nc.sync.dma_start(out=xT_dst[:, 0:1, 0:512], in_=xT_src[:, 0:1, 0:512])
        nc.sync.dma_start(out=xT_dst[:, 1:N_DK, 0:512], in_=xT_src[:, 1:N_DK, 0:512])
        for lo_, hi_ in wq_groups[1:]:
            nc.sync.dma_start(out=wqkv_dst[:, :, lo_:hi_], in_=wqkv_src[:, :, lo_:hi_])
        # K^T zero-padded per head: head h of pair p occupies its 64 rows.
        qT = [sb.tile([128, L], B16, tag=f"qT{p}", bufs=1, name=f"qT{p}") for p in range(2)]
        kz = [
            [
                sb.tile([128, L], B16, tag=f"kz{p}{h}", bufs=1, name=f"kz{p}{h}")
                for h in range(2)
            ]
            for p in range(2)
        ]
        for p in range(2):
            nc.sync.dma_start(out=kz[p][0][64:128, :], in_=zer_d[:])
            nc.sync.dma_start(out=kz[p][1][0:64, :], in_=zer_d[:])
        for s in range(1, NS):
            nc.sync.dma_start(
                out=xT_dst[:, :, 512 * s : 512 * (s + 1)],
                in_=xT_src[:, :, 512 * s : 512 * (s + 1)],
            )
        wproj_t = sb.tile([128, 2 * D], B16, tag="wproj", bufs=1, name="wproj")
        wproj = [wproj_t[:, D * kt : D * (kt + 1)] for kt in range(2)]
        nc.sync.dma_start(
            out=wproj_t[:].rearrange("p (k c) -> p k c", k=2),
            in_=wproj_d.rearrange("(k p) c -> p k c", k=2),
        )
        # V in natural [tok, vcol] layout per k-tile; head hg occupies columns
        # 65*hg..65*hg+64 as [v (64) | 1]; the ones column carries Z.
        vsb = [sb.tile([128, VW], B16, tag=f"v{t}", bufs=1, name=f"v{t}") for t in range(KT)]
        attnT = [
            sb.tile([128, L], B16, tag=f"attnT{k}", bufs=1, name=f"attnT{k}")
            for k in range(2)
        ]

        def mmt(out, lhsT, rhs, cols, **kw):
            mm(out, lhsT, rhs, **kw)
            E["pe"] += cols / 2.4 + 25.0
            if E["in_fill"]:
                E["fill"] += cols / 2.4 + 25.0

        # ---- emitters (generators yield after each PE-significant unit) ----
        def gen_qkv(s):
            cs = slice(512 * s, 512 * (s + 1))
            for m in range(4):
                p_qk = ps.tile([128, 512], F32, tag="mm", bufs=2, name="p_qk")
                for k in range(N_DK):
                    mmt(
                        p_qk[:],
                        wqkv[k][:, 128 * m : 128 * (m + 1)],
                        xTc[k][s][:],
                        cols=512,
                        start=(k == 0),
                        stop=(k == N_DK - 1),
                    )
                    yield
                if m < 2:
                    nc.vector.tensor_scalar_add(qT[m][:, cs], p_qk[:], bqk[:, m : m + 1])
                else:
                    p = m - 2
                    nc.vector.tensor_scalar_add(
                        kz[p][0][0:64, cs], p_qk[0:64, :], bqk[0:64, m : m + 1]
                    )
                    nc.vector.tensor_scalar_add(
                        kz[p][1][64:128, cs], p_qk[64:128, :], bqk[64:128, m : m + 1]
                    )
                yield
            for j in range(4):
                t = 4 * s + j
                p_v = ps.tile([128, VW], F32, tag="mm", bufs=2, name="p_v")
                for k in range(N_DK):
                    mmt(
                        p_v[:],
                        xTc[k][s][:, 128 * j : 128 * (j + 1)],
                        wqkv[k][:, 2 * CD : 2 * CD + VW],
                        cols=VW,
                        start=(k == 0),
                        stop=False,
                    )
                    yield
                mmt(p_v[:], onesb[:], bv[:], cols=VW, start=False, stop=True)
                nc.vector.tensor_copy(vsb[t][:], p_v[:])
                yield

        def gen_proj(s):
            # bproj is added on the host (linear past the partial-sum
            # reduction); the psum->sbuf cast alternates ACT/DVE
            cs = slice(512 * s, 512 * (s + 1))
            for m in range(N_DK):
                p_y = ps.tile([128, 512], F32, tag="mm", bufs=2, name="p_y")
                for kt in range(2):
                    mmt(
                        p_y[:],
                        wproj[kt][:, 128 * m : 128 * (m + 1)],
                        attnT[kt][:, cs],
                        cols=512,
                        start=(kt == 0),
                        stop=(kt == 1),
                    )
                    yield
                y_sb = sb.tile([128, 512], B16, tag="ysb", bufs=4)
                if m % 2 == 0:
                    nc.vector.tensor_copy(y_sb[:], p_y[:])
                else:
                    nc.scalar.activation(y_sb[:], p_y[:], IDF)
                    E["act"] += (512 + 352) / 1.2
                nc.sync.dma_start(out=yT_d[128 * m : 128 * (m + 1), cs], in_=y_sb[:])
                yield

        class Gen:
            def __init__(self, g):
                self.g = g
                self.done = False

            def step(self):
                if self.done:
                    return False
                try:
                    next(self.g)
                    return True
                except StopIteration:
                    self.done = True
                    return False

        fillers = deque()

        def pull():
            E["in_fill"] = True
            try:
                while fillers:
                    if fillers[0].step():
                        return True
                    fillers.popleft()
                return False
            finally:
                E["in_fill"] = False

        # Pace fillers PROPORTIONALLY to attention-phase ACT progress so all
        # filler work is emitted by the time attention ends (a trailing
        # balance would dump most of proj into a serialized tail).
        FILL_TOTAL = (NS - 1) * (
            4 * N_DK * (512 / 2.4 + 25.0) + 4 * (N_DK + 1) * (VW / 2.4 + 25.0)
        ) + NS * N_DK * 2 * (512 / 2.4 + 25.0)
        ACT_TOTAL = (69632 + 80 * 352) / 1.2 + 48 * (512 + 352) / 1.2

        def balance():
            while E["fill"] < FILL_TOTAL * min(1.0, E["act"] / ACT_TOTAL) and pull():
                pass

        def emit_norm(pair, s, av0, av1):
            cs = slice(512 * s, 512 * (s + 1))
            bcs = sb.tile([64, 1024], F32, tag="bcs", bufs=2, name="bcs")
            for h, av in ((0, av0), (1, av1)):
                # 1/Z = exp(-ln Z) on ACT: ln/exp/identity live in ONE table
                # set (natural_log_exp_and_others) so no table switches.
                lnz = sb.tile([1, 512], F32, tag="lnz", bufs=4, name="lnz")
                nc.scalar.activation(lnz[:], av[64:65, :], LNF)
                E["act"] += (512 + 352) / 1.2
                rz = sb.tile([1, 512], R32, tag="rz", bufs=4, name="rz")
                nc.scalar.activation(rz[:], lnz[:], EXPF, scale=-1.0)
                E["act"] += (512 + 352) / 1.2
                bc_ps = ps.tile([64, 512], F32, tag="mm", bufs=2, name="bc_ps")
                mmt(bc_ps[:], onesr[:], rz[:], cols=512, start=True, stop=True)
                nc.vector.tensor_copy(bcs[:, 512 * h : 512 * (h + 1)], bc_ps[:])
            nc.vector.tensor_tensor(
                attnT[pair][0:64, cs], av0[0:64, :], bcs[:, 0:512], op=MULT
            )
            tmp = sb.tile([64, 512], B16, tag="ntmp", bufs=2, name="tmp")
            nc.vector.tensor_tensor(tmp[:], av1[0:64, :], bcs[:, 512:1024], op=MULT)
            nc.sync.dma_start(out=attnT[pair][64:128, cs], in_=tmp[:])

        def warmup(n):
            # dummy [128,128] matmuls into a scratch psum: keep the PE busy
            # through front DMA-wait gaps so the HAM clock-gate warms up
            # (~3.4us of sustained activity unlocks 2.4 GHz) instead of
            # idling cold through the whole QKV(0) phase
            wps = ps.tile([128, 128], F32, tag="st", bufs=2, name="warm")
            for _ in range(n):
                mmt(wps[:], negi[:], u128[:], cols=128, start=True, stop=True)

        # ================= QKV chunk 0 (+ warmup filler) =================
        g0 = gen_qkv(0)
        warmup(12)
        for i, _ in enumerate(g0):
            # m-tiles yield 9 units each (8 mms + copy); dummy-fill between
            # the early mms where the front x/weight DMAs are still landing
            if i == 1:
                warmup(12)
            elif i in (9, 18):
                warmup(6)
        qkv_gens = {c: Gen(gen_qkv(c)) for c in range(1, NS)}
        fillers.extend(qkv_gens.values())

        # ================= attention (+ interleaved fillers) =================
        pend = []  # (bid, av_mm_args, kw, cols)
        fin_prev = None  # (bid, pair, s, av0, av1)
        for s in range(NS):
            q0 = 512 * s
            # the PE runs in program order: every QKV(s) producer must be
            # EMITTED before attention reads qT/kz/vsb of chunk s, or the
            # score matmul deadlocks waiting on a producer queued behind it
            if s >= 1:
                while not qkv_gens[s].done:
                    pull()
            for pair in range(2):
                bid = 2 * s + pair
                nk = 4 * s + 4
                av0 = ps.tile([128, 512], F32, tag="av", bufs=2, name="av0")
                av1 = ps.tile([128, 512], F32, tag="av", bufs=2, name="av1")
                for k in range(nk):
                    if k == 1 and fin_prev is not None:
                        pbid = fin_prev[0]
                        while pend and pend[0][0] == pbid:
                            _, a, kw, w = pend.pop(0)
                            mmt(*a, cols=w, **kw)
                        emit_norm(*fin_prev[1:])
                        if fin_prev[1] == 1:  # pair 1 normed -> proj(s) ready
                            fillers.append(Gen(gen_proj(fin_prev[2])))
                        fin_prev = None
                    k0 = 128 * k
                    dt_ = k - 4 * s
                    lo = 128 * dt_ if dt_ >= 0 else 0
                    st = ps.tile([128, 1024], F32, tag="st", bufs=2, name="st")
                    for h in range(2):
                        mmt(
                            st[:, 512 * h + lo : 512 * h + 512],
                            kz[pair][h][:, k0 : k0 + 128],
                            qT[pair][:, q0 + lo : q0 + 512],
                            cols=512 - lo,
                            start=True,
                            stop=(dt_ < 0),
                        )
                    if dt_ >= 0:
                        for h in range(2):
                            mmt(
                                st[:, 512 * h + lo : 512 * h + lo + 128],
                                negi[:],
                                u128[:],
                                cols=128,
                                start=False,
                                stop=True,
                                skip_group_check=True,
                            )
                    pt = sb.tile([128, 1024], B16, tag="pt", bufs=AV_DELAY + 3, name="pt")
                    if lo == 0:
                        nc.scalar.activation(pt[:], st[:], EXPF, scale=SCALE)
                        E["act"] += (1024 + 352) / 1.2
                    else:
                        stv = st[:].rearrange("p (h q) -> p h q", h=2)[:, :, lo:512]
                        ptv = pt[:].rearrange("p (h q) -> p h q", h=2)[:, :, lo:512]
                        nc.scalar.activation(ptv, stv, EXPF, scale=SCALE)
                        E["act"] += (2 * (512 - lo) + 352) / 1.2
                    vb = 130 * pair
                    pend.append(
                        (
                            bid,
                            (av0[0:65, lo:512], vsb[k][:, vb : vb + 65], pt[:, lo:512]),
                            dict(start=(k == 0), stop=(k == nk - 1), skip_group_check=True),
                            512 - lo,
                        )
                    )
                    pend.append(
                        (
                            bid,
                            (
                                av1[0:65, lo:512],
                                vsb[k][:, vb + 65 : vb + 130],
                                pt[:, 512 + lo : 1024],
                            ),
                            dict(start=(k == 0), stop=(k == nk - 1), skip_group_check=True),
                            512 - lo,
                        )
                    )
                    while len(pend) > 2 * AV_DELAY:
                        _, a, kw, w = pend.pop(0)
                        mmt(*a, cols=w, **kw)
                    balance()
                fin_prev = (bid, pair, s, av0, av1)
        while pend:
            _, a, kw, w = pend.pop(0)
            mmt(*a, cols=w, **kw)
        emit_norm(*fin_prev[1:])
        fillers.append(Gen(gen_proj(NS - 1)))
        while pull():
            pass
    _split_multi_waits(nc)
    return nc


_NC_CACHE = None
LAST_RESULTS = None

_ONESB = np.ones((1, 128), dtype=NPB16)
_ONESR = np.ones((1, 64), dtype=np.float32)
_I, _J = np.meshgrid(np.arange(128), np.arange(128), indexing="ij")
_NEGI = (np.where(_I == _J, NEG, 0.0)).astype(NPB16)
_U128 = (np.where(_I > _J, 1.0, 0.0)).astype(NPB16)
_ZER = np.zeros((64, L), dtype=NPB16)


def _make_in_maps(x, Wqkv, bqkv, Wproj, bproj):
    in_maps = []
    for c in range(N_CORES):
        b, g = divmod(c, 4)
        qc = slice(CD * g, CD * (g + 1))
        wq = Wqkv[:, qc]
        wk = Wqkv[:, D : 2 * D][:, qc]
        wv = Wqkv[:, 2 * D : 3 * D][:, qc]
        bq = bqkv[qc]
        bk = bqkv[D : 2 * D][qc]
        bvv = bqkv[2 * D : 3 * D][qc]
        # V columns interleaved per head: [wv_h (64 cols) | zeros col]; the
        # ones column (zero weight col + 1.0 bias) carries the row-sum Z.
        wv_i = np.zeros((D, VW), dtype=np.float32)
        bv_i = np.zeros((1, VW), dtype=np.float32)
        for h in range(HPC):
            wv_i[:, 65 * h : 65 * h + 64] = wv[:, 64 * h : 64 * h + 64]
            bv_i[0, 65 * h : 65 * h + 64] = bvv[64 * h : 64 * h + 64]
            bv_i[0, 65 * h + 64] = 1.0
        bqk_cols = np.concatenate([bq, bk]).reshape(4, 128).T  # [128, 4]
        in_maps.append(
            {
                "xT": np.ascontiguousarray(x[b].T.astype(NPB16)),
                "wqkv": np.ascontiguousarray(
                    np.concatenate([wq, wk, wv_i], axis=1).astype(NPB16)
                ),
                "bqk": np.ascontiguousarray(bqk_cols),
                "bv": bv_i.astype(NPB16),
                "wproj": np.ascontiguousarray(
                    Wproj[CD * g : CD * (g + 1), :].astype(NPB16)
                ),
                "onesb": _ONESB,
                "onesr": _ONESR,
                "negi": _NEGI,
                "u128": _U128,
                "zer": _ZER,
            }
        )

    return in_maps


def kernel(x, Wqkv, bqkv, Wproj, bproj):
    global _NC_CACHE, LAST_RESULTS
    x = np.asarray(x, dtype=np.float32)
    Wqkv = np.asarray(Wqkv, dtype=np.float32)
    bqkv = np.asarray(bqkv, dtype=np.float32)
    Wproj = np.asarray(Wproj, dtype=np.float32)
    bproj = np.asarray(bproj, dtype=np.float32)

    if _NC_CACHE is None:
        _NC_CACHE = _build_program()
    nc = _NC_CACHE

    in_maps = _make_in_maps(x, Wqkv, bqkv, Wproj, bproj)
    res = run_bass_kernel_spmd(nc, in_maps, core_ids=list(range(N_CORES)))
    LAST_RESULTS = res

    out = np.empty((B, L, D), dtype=np.float32)
    for b in range(B):
        acc = res.results[4 * b]["yT"].astype(np.float32)
        for g in range(1, 4):
            acc = acc + res.results[4 * b + g]["yT"].astype(np.float32)
        out[b] = acc.T + bproj[None, :]
    return out


# revision 40
# speedup vs baseline: 1.0358x; 1.0358x over previous
"""Causal self-attention (B=2, L=2048, D=1024, H=16) on 8 trn2 NeuronCores.

Sharding: core c = 4*b + g handles batch b and head group g (4 heads).
Per core: QKV projection for its heads' weight columns (tensor-parallel),
flash-style causal attention for its 4 heads, and a partial output
projection over its 256 head-dims (row-parallel).  The host sums the 4
partial projections per batch and adds bproj.

v2 design (single interleaved instruction stream, all-bf16 matmuls):
  - All matmul operands bf16 (FWL weight loads, half DMA); psum stays f32.
  - One software-pipelined stream: QKV chunk s+1 and output-projection
    chunk s-1 matmuls are pulled as PE "filler" between attention steps,
    paced by an emitted-time estimator, so the PE never idles while the
    scalar engine grinds through exps (ACT is the attention-phase
    co-bottleneck at ~1 elem/lane/cycle + ~350cy/call overhead).
  - Score psums are [128, 1024] (2 banks): both heads of a pair per
    k-tile, one merged exp call per step (halves ACT call overhead).
  - Causal diagonal masked ON THE PE: an extra accumulation matmul
    negi^T @ u128 adds -1e30 above the diagonal (no DVE mask adds).
  - AV rides the ones-column trick for Z (col 64 of each head group).
  - Normalize: 1/Z via single-pass custom-DVE reciprocal_approx_fast
    (~51 ULP) into an f32r tile, broadcast across partitions with a
    K=1 ones matmul, psum->sbuf copies, then in-partition multiplies
    reading the AV psum directly (no separate un-normalized copy);
    head1's half goes through a tmp tile + SBUF->SBUF DMA since matmul
    psum bases are restricted to partitions 0/32/64.
  - QKV/V copies out of psum moved to the vector engine (tensor_scalar
    bias-add fused); y copies stay on ACT (bias fused), output bf16.
"""

import sys
import types
from collections import deque

import numpy as np


def _install_ntff_shim():
    """The container's antenv stub lacks axon_hooks; recreate it so
    run_bass_kernel_spmd(trace=True) can reach the NTFF profiler."""
    if "antenv.axon_hooks" in sys.modules:
        return
    try:
        import antenv
        from trn_agent_boot.trn_boot import _ntff_profile_via_ctypes
    except Exception:
        return
    mod = types.ModuleType("antenv.axon_hooks")
    hook = _ntff_profile_via_ctypes("/opt/axon/libaxon_pjrt.so")
    mod.get_axon_ntff_profile_hook = lambda: hook
    mod.set_axon_ntff_profile_hook = lambda h: None
    sys.modules["antenv.axon_hooks"] = mod
    antenv.axon_hooks = mod


_install_ntff_shim()

import ml_dtypes  # noqa: E402

import concourse.bass as bass  # noqa: E402
import concourse.mybir as mybir  # noqa: E402
import concourse.tile as tile  # noqa: E402
from concourse.bass_utils import run_bass_kernel_spmd  # noqa: E402
from concourse.vector_clock import ScopedClock, VectorClock  # noqa: E402

B, L, D, H = 2, 2048, 1024, 16
HD = D // H  # 64
N_CORES = 8
HPC = 4  # heads per core
CD = HPC * HD  # 256 head-dims per core
VW = HPC * (HD + 1)  # 260 interleaved V columns
SCALE = HD**-0.5  # 0.125
F32 = mybir.dt.float32
R32 = mybir.dt.float32r
B16 = mybir.dt.bfloat16
NPB16 = ml_dtypes.bfloat16
NEG = -1.0e30

KT = L // 128  # 16 k-tiles of 128 keys
NS = L // 512  # 4 query chunks of 512
N_DK = D // 128  # 8 feature k-tiles
AV_DELAY = 4  # AV matmuls issue this many merged steps behind their exp
EXPF = mybir.ActivationFunctionType.Exp
IDF = mybir.ActivationFunctionType.Identity
LNF = mybir.ActivationFunctionType.Ln
MULT = mybir.AluOpType.mult


class _TileContext(tile.TileContext):
    """Split exit-drain sem waits to 1 per drain; this walrus build's
    CTRL codegen rejects drains with 2+ sync waits."""

    def _drain_and_barrier(self, tick_clock, wait_clock):
        g = tick_clock.global_clock
        n = len(g)
        procs = [i for i in range(n) if g[i] > 0]
        for p in procs:
            vec = [g[i] if i == p else 0 for i in range(n)]
            d = self.nc.sync.drain()
            wait_clock.add_sem_waits(d.ins, ScopedClock({None: VectorClock(vec)}))
        self.nc.all_engine_barrier()
        popped = self.nc._tile_sem_poison_stack.pop()
        assert popped is self._sem_poison
        self.nc.clear_and_free_semaphores(list(self.sems.allocated().values()))
        self.nc.all_engine_barrier()


def _split_multi_waits(nc):
    """This walrus build's codegen accepts only ONE sync wait per
    instruction; hoist extra waits onto preceding same-engine NOPs."""
    for f in nc.m.functions:
        for blk in f.blocks:
            orig = list(blk.instructions)
            expanded = []
            changed = False
            for ins in orig:
                si = ins.sync_info
                if si is not None and si.on_wait is not None and len(si.on_wait) > 1:
                    changed = True
                    waits = list(si.on_wait)
                    eng = nc.engines[ins.engine]
                    for w in waits[:-1]:
                        nop = eng.nop(nofuse=True).ins
                        nc.cur_bb.bb.instructions.remove(nop)
                        nop.sync_info = mybir.SyncInfo(on_wait=[w], on_update=[])
                        expanded.append(nop)
                    ins.sync_info = mybir.SyncInfo(
                        on_wait=[waits[-1]], on_update=list(si.on_update or [])
                    )
                expanded.append(ins)
            if changed:
                il = blk.instructions
                for ins in list(il):
                    il.remove(ins)
                for ins in expanded:
                    il.append(ins)


def _build_program():
    nc = bass.Bass()
    xT_d = nc.dram_tensor("xT", [D, L], B16, kind="ExternalInput").ap()
    wqkv_d = nc.dram_tensor("wqkv", [D, 2 * CD + VW], B16, kind="ExternalInput").ap()
    bqk_d = nc.dram_tensor("bqk", [128, 4], F32, kind="ExternalInput").ap()
    bv_d = nc.dram_tensor("bv", [1, VW], B16, kind="ExternalInput").ap()
    wproj_d = nc.dram_tensor("wproj", [CD, D], B16, kind="ExternalInput").ap()
    onesb_d = nc.dram_tensor("onesb", [1, 128], B16, kind="ExternalInput").ap()
    onesr_d = nc.dram_tensor("onesr", [1, 64], R32, kind="ExternalInput").ap()
    negi_d = nc.dram_tensor("negi", [128, 128], B16, kind="ExternalInput").ap()
    u128_d = nc.dram_tensor("u128", [128, 128], B16, kind="ExternalInput").ap()
    zer_d = nc.dram_tensor("zer", [64, L], B16, kind="ExternalInput").ap()
    yT_d = nc.dram_tensor("yT", [D, L], B16, kind="ExternalOutput").ap()

    mm = nc.tensor.matmul
    E = {"pe": 0.0, "act": 0.0, "fill": 0.0, "in_fill": False}

    with _TileContext(nc) as tc, tc.tile_pool(name="sb", bufs=1) as sb, tc.tile_pool(
        name="ps", bufs=1, space="PSUM"
    ) as ps, nc.allow_low_precision(reason="bf16 attention kernel"):
        # ---- constants (issued on the ACT HWDGE ring: the sync ring's
        # trigger FIFO (~0.6us each) must stay clear for the critical
        # weight/activation loads) ----
        negi = sb.tile([128, 128], B16, tag="negi", bufs=1)
        nc.scalar.dma_start(out=negi[:], in_=negi_d[:])
        u128 = sb.tile([128, 128], B16, tag="u128", bufs=1)
        nc.scalar.dma_start(out=u128[:], in_=u128_d[:])
        onesb = sb.tile([1, 128], B16, tag="onesb", bufs=1)
        nc.scalar.dma_start(out=onesb[:], in_=onesb_d[:])
        onesr = sb.tile([1, 64], R32, tag="onesr", bufs=1)
        nc.scalar.dma_start(out=onesr[:], in_=onesr_d[:])
        bqk = sb.tile([128, 4], F32, tag="bqk", bufs=1)
        nc.scalar.dma_start(out=bqk[:], in_=bqk_d[:])
        bv = sb.tile([1, VW], B16, tag="bv", bufs=1)
        nc.scalar.dma_start(out=bv[:], in_=bv_d[:])

        # ---- persistent SBUF tensors + DMA schedule ----
        # One big tile per tensor with per-k column views: a single batched
        # DMA per group (one ~0.6us sync-ring trigger instead of 8).
        WQW = 2 * CD + VW  # 772
        wqkv_t = sb.tile([128, N_DK * WQW], B16, tag="wqkv", bufs=1, name="wqkv")
        wqkv = [wqkv_t[:, WQW * k : WQW * (k + 1)] for k in range(N_DK)]
        wqkv_src = wqkv_d.rearrange("(k p) c -> p k c", k=N_DK)
        wqkv_dst = wqkv_t[:].rearrange("p (k c) -> p k c", k=N_DK)
        xT_t = sb.tile([128, N_DK * L], B16, tag="xT", bufs=1, name="xT")
        xTc = [
            [xT_t[:, L * k + 512 * s : L * k + 512 * (s + 1)] for s in range(NS)]
            for k in range(N_DK)
        ]
        xT_src = xT_d.rearrange("(k p) c -> p k c", k=N_DK)
        xT_dst = xT_t[:].rearrange("p (k c) -> p k c", k=N_DK)
        # first-consumption order: w cols 0..255 (m0/m1), x chunk0 for k=0,
        # then the rest of x chunk0, w 256..511, V cols, zero-halves, x 1-3
        wq_groups = [(0, 256), (256, 512), (512, WQW)]
        lo_, hi_ = wq_groups[0]
        nc.sync.dma_start(out=wqkv_dst[:, :, lo_:hi_], in_=wqkv_src[:, :, lo_:hi_])
        nc.sync.dma_start(out=xT_dst[:, 0:1, 0:512], in_=xT_src[:, 0:1, 0:512])
        nc.sync.dma_start(out=xT_dst[:, 1:N_DK, 0:512], in_=xT_src[:, 1:N_DK, 0:512])
        for lo_, hi_ in wq_groups[1:]:
            nc.sync.dma_start(out=wqkv_dst[:, :, lo_:hi_], in_=wqkv_src[:, :, lo_:hi_])
        # K^T zero-padded per head: head h of pair p occupies its 64 rows.
        qT = [sb.tile([128, L], B16, tag=f"qT{p}", bufs=1, name=f"qT{p}") for p in range(2)]
        kz = [
            [
                sb.tile([128, L], B16, tag=f"kz{p}{h}", bufs=1, name=f"kz{p}{h}")
                for h in range(2)
            ]
            for p in range(2)
        ]
        for p in range(2):
            nc.sync.dma_start(out=kz[p][0][64:128, :], in_=zer_d[:])
            nc.sync.dma_start(out=kz[p][1][0:64, :], in_=zer_d[:])
        for s in range(1, NS):
            nc.sync.dma_start(
                out=xT_dst[:, :, 512 * s : 512 * (s + 1)],
                in_=xT_src[:, :, 512 * s : 512 * (s + 1)],
            )
        wproj_t = sb.tile([128, 2 * D], B16, tag="wproj", bufs=1, name="wproj")
        wproj = [wproj_t[:, D * kt : D * (kt + 1)] for kt in range(2)]
        nc.sync.dma_start(
            out=wproj_t[:].rearrange("p (k c) -> p k c", k=2),
            in_=wproj_d.rearrange("(k p) c -> p k c", k=2),
        )
        # V in natural [tok, vcol] layout per k-tile; head hg occupies columns
        # 65*hg..65*hg+64 as [v (64) | 1]; the ones column carries Z.
        vsb = [sb.tile([128, VW], B16, tag=f"v{t}", bufs=1, name=f"v{t}") for t in range(KT)]
        attnT = [
            sb.tile([128, L], B16, tag=f"attnT{k}", bufs=1, name=f"attnT{k}")
            for k in range(2)
        ]

        def mmt(out, lhsT, rhs, cols, **kw):
            mm(out, lhsT, rhs, **kw)
            E["pe"] += cols / 2.4 + 25.0
            if E["in_fill"]:
                E["fill"] += cols / 2.4 + 25.0

        # ---- emitters (generators yield after each PE-significant unit) ----
        def gen_qkv(s):
            cs = slice(512 * s, 512 * (s + 1))
            for m in range(4):
                p_qk = ps.tile([128, 512], F32, tag="mm", bufs=2, name="p_qk")
                for k in range(N_DK):
                    mmt(
                        p_qk[:],
                        wqkv[k][:, 128 * m : 128 * (m + 1)],
                        xTc[k][s][:],
                        cols=512,
                        start=(k == 0),
                        stop=(k == N_DK - 1),
                    )
                    yield
                if m < 2:
                    nc.vector.tensor_scalar_add(qT[m][:, cs], p_qk[:], bqk[:, m : m + 1])
                else:
                    p = m - 2
                    nc.vector.tensor_scalar_add(
                        kz[p][0][0:64, cs], p_qk[0:64, :], bqk[0:64, m : m + 1]
                    )
                    nc.vector.tensor_scalar_add(
                        kz[p][1][64:128, cs], p_qk[64:128, :], bqk[64:128, m : m + 1]
                    )
                yield
            for j in range(4):
                t = 4 * s + j
                p_v = ps.tile([128, VW], F32, tag="mm", bufs=2, name="p_v")
                for k in range(N_DK):
                    mmt(
                        p_v[:],
                        xTc[k][s][:, 128 * j : 128 * (j + 1)],
                        wqkv[k][:, 2 * CD : 2 * CD + VW],
                        cols=VW,
                        start=(k == 0),
                        stop=False,
                    )
                    yield
                mmt(p_v[:], onesb[:], bv[:], cols=VW, start=False, stop=True)
                nc.vector.tensor_copy(vsb[t][:], p_v[:])
                yield

        def gen_proj(s):
            # bproj is added on the host (linear past the partial-sum
            # reduction); the psum->sbuf cast alternates ACT/DVE
            cs = slice(512 * s, 512 * (s + 1))
            for m in range(N_DK):
                p_y = ps.tile([128, 512], F32, tag="mm", bufs=2, name="p_y")
                for kt in range(2):
                    mmt(
                        p_y[:],
                        wproj[kt][:, 128 * m : 128 * (m + 1)],
                        attnT[kt][:, cs],
                        cols=512,
                        start=(kt == 0),
                        stop=(kt == 1),
                    )
                    yield
                y_sb = sb.tile([128, 512], B16, tag="ysb", bufs=4)
                if m % 2 == 0:
                    nc.vector.tensor_copy(y_sb[:], p_y[:])
                else:
                    nc.scalar.activation(y_sb[:], p_y[:], IDF)
                    E["act"] += (512 + 352) / 1.2
                nc.sync.dma_start(out=yT_d[128 * m : 128 * (m + 1), cs], in_=y_sb[:])
                yield

        class Gen:
            def __init__(self, g):
                self.g = g
                self.done = False

            def step(self):
                if self.done:
                    return False
                try:
                    next(self.g)
                    return True
                except StopIteration:
                    self.done = True
                    return False

        fillers = deque()

        def pull():
            E["in_fill"] = True
            try:
                while fillers:
                    if fillers[0].step():
                        return True
                    fillers.popleft()
                return False
            finally:
                E["in_fill"] = False

        # Pace fillers PROPORTIONALLY to attention-phase ACT progress so all
        # filler work is emitted by the time attention ends (a trailing
        # balance would dump most of proj into a serialized tail).
        FILL_TOTAL = (NS - 1) * (
            4 * N_DK * (512 / 2.4 + 25.0) + 4 * (N_DK + 1) * (VW / 2.4 + 25.0)
        ) + NS * N_DK * 2 * (512 / 2.4 + 25.0)
        ACT_TOTAL = (69632 + 80 * 352) / 1.2 + 48 * (512 + 352) / 1.2

        def balance():
            while E["fill"] < FILL_TOTAL * min(1.0, E["act"] / ACT_TOTAL) and pull():
                pass

        def emit_norm(pair, s, av0, av1):
            # 1/Z = exp(-ln Z) on ACT: ln/exp/identity live in ONE table set
            # (natural_log_exp_and_others) so no table switches.  Ops are
            # batched per engine (both heads' lns, then both exps, ...) to
            # halve the cross-engine serial latency when this chain is
            # exposed at the end of the kernel.
            cs = slice(512 * s, 512 * (s + 1))
            bcs = sb.tile([64, 1024], F32, tag="bcs", bufs=2, name="bcs")
            lnzs, rzs = [], []
            for h, av in ((0, av0), (1, av1)):
                lnz = sb.tile([1, 512], F32, tag="lnz", bufs=4, name="lnz")
                nc.scalar.activation(lnz[:], av[64:65, :], LNF)
                E["act"] += (512 + 352) / 1.2
                lnzs.append(lnz)
            for h in range(2):
                rz = sb.tile([1, 512], R32, tag="rz", bufs=4, name="rz")
                nc.scalar.activation(rz[:], lnzs[h][:], EXPF, scale=-1.0)
                E["act"] += (512 + 352) / 1.2
                rzs.append(rz)
            bc_list = []
            for h in range(2):
                bc_ps = ps.tile([64, 512], F32, tag="mm", bufs=2, name="bc_ps")
                mmt(bc_ps[:], onesr[:], rzs[h][:], cols=512, start=True, stop=True)
                bc_list.append(bc_ps)
            for h in range(2):
                nc.vector.tensor_copy(bcs[:, 512 * h : 512 * (h + 1)], bc_list[h][:])
            nc.vector.tensor_tensor(
                attnT[pair][0:64, cs], av0[0:64, :], bcs[:, 0:512], op=MULT
            )
            tmp = sb.tile([64, 512], B16, tag="ntmp", bufs=2, name="tmp")
            nc.vector.tensor_tensor(tmp[:], av1[0:64, :], bcs[:, 512:1024], op=MULT)
            nc.sync.dma_start(out=attnT[pair][64:128, cs], in_=tmp[:])

        def warmup(n):
            # dummy [128,128] matmuls into a scratch psum: keep the PE busy
            # through front DMA-wait gaps so the HAM clock-gate warms up
            # (~3.4us of sustained activity unlocks 2.4 GHz) instead of
            # idling cold through the whole QKV(0) phase
            wps = ps.tile([128, 128], F32, tag="st", bufs=2, name="warm")
            for _ in range(n):
                mmt(wps[:], negi[:], u128[:], cols=128, start=True, stop=True)

        # ================= QKV chunk 0 =================
        for _ in gen_qkv(0):
            pass
        qkv_gens = {c: Gen(gen_qkv(c)) for c in range(1, NS)}
        fillers.extend(qkv_gens.values())

        # ================= attention (+ interleaved fillers) =================
        pend = []  # (bid, av_mm_args, kw, cols)
        fin_prev = None  # (bid, pair, s, av0, av1)
        for s in range(NS):
            q0 = 512 * s
            # the PE runs in program order: every QKV(s) producer must be
            # EMITTED before attention reads qT/kz/vsb of chunk s, or the
            # score matmul deadlocks waiting on a producer queued behind it
            if s >= 1:
                while not qkv_gens[s].done:
                    pull()
            for pair in range(2):
                bid = 2 * s + pair
                nk = 4 * s + 4
                av0 = ps.tile([128, 512], F32, tag="av", bufs=2, name="av0")
                av1 = ps.tile([128, 512], F32, tag="av", bufs=2, name="av1")
                for k in range(nk):
                    if k == 1 and fin_prev is not None:
                        pbid = fin_prev[0]
                        while pend and pend[0][0] == pbid:
                            _, a, kw, w = pend.pop(0)
                            mmt(*a, cols=w, **kw)
                        emit_norm(*fin_prev[1:])
                        if fin_prev[1] == 1:  # pair 1 normed -> proj(s) ready
                            fillers.append(Gen(gen_proj(fin_prev[2])))
                        fin_prev = None
                    k0 = 128 * k
                    dt_ = k - 4 * s
                    lo = 128 * dt_ if dt_ >= 0 else 0
                    st = ps.tile([128, 1024], F32, tag="st", bufs=2, name="st")
                    for h in range(2):
                        mmt(
                            st[:, 512 * h + lo : 512 * h + 512],
                            kz[pair][h][:, k0 : k0 + 128],
                            qT[pair][:, q0 + lo : q0 + 512],
                            cols=512 - lo,
                            start=True,
                            stop=(dt_ < 0),
                        )
                    if dt_ >= 0:
                        for h in range(2):
                            mmt(
                                st[:, 512 * h + lo : 512 * h + lo + 128],
                                negi[:],
                                u128[:],
                                cols=128,
                                start=False,
                                stop=True,
                                skip_group_check=True,
                            )
                    pt = sb.tile([128, 1024], B16, tag="pt", bufs=AV_DELAY + 3, name="pt")
                    if lo == 0:
                        nc.scalar.activation(pt[:], st[:], EXPF, scale=SCALE)
                        E["act"] += (1024 + 352) / 1.2
                    else:
                        stv = st[:].rearrange("p (h q) -> p h q", h=2)[:, :, lo:512]
                        ptv = pt[:].rearrange("p (h q) -> p h q", h=2)[:, :, lo:512]
                        nc.scalar.activation(ptv, stv, EXPF, scale=SCALE)
                        E["act"] += (2 * (512 - lo) + 352) / 1.2
                    vb = 130 * pair
                    pend.append(
                        (
                            bid,
                            (av0[0:65, lo:512], vsb[k][:, vb : vb + 65], pt[:, lo:512]),
                            dict(start=(k == 0), stop=(k == nk - 1), skip_group_check=True),
                            512 - lo,
                        )
                    )
                    pend.append(
                        (
                            bid,
                            (
                                av1[0:65, lo:512],
                                vsb[k][:, vb + 65 : vb + 130],
                                pt[:, 512 + lo : 1024],
                            ),
                            dict(start=(k == 0), stop=(k == nk - 1), skip_group_check=True),
                            512 - lo,
                        )
                    )
                    while len(pend) > 2 * AV_DELAY:
                        _, a, kw, w = pend.pop(0)
                        mmt(*a, cols=w, **kw)
                    balance()
                fin_prev = (bid, pair, s, av0, av1)
        while pend:
            _, a, kw, w = pend.pop(0)
            mmt(*a, cols=w, **kw)
        emit_norm(*fin_prev[1:])
        fillers.append(Gen(gen_proj(NS - 1)))
        while pull():
            pass
    _split_multi_waits(nc)
    return nc


_NC_CACHE = None
LAST_RESULTS = None

_ONESB = np.ones((1, 128), dtype=NPB16)
_ONESR = np.ones((1, 64), dtype=np.float32)
_I, _J = np.meshgrid(np.arange(128), np.arange(128), indexing="ij")
_NEGI = (np.where(_I == _J, NEG, 0.0)).astype(NPB16)
_U128 = (np.where(_I > _J, 1.0, 0.0)).astype(NPB16)
_ZER = np.zeros((64, L), dtype=NPB16)


def _make_in_maps(x, Wqkv, bqkv, Wproj, bproj):
    in_maps = []
    for c in range(N_CORES):
        b, g = divmod(c, 4)
        qc = slice(CD * g, CD * (g + 1))
        wq = Wqkv[:, qc]
        wk = Wqkv[:, D : 2 * D][:, qc]
        wv = Wqkv[:, 2 * D : 3 * D][:, qc]
        bq = bqkv[qc]
        bk = bqkv[D : 2 * D][qc]
        bvv = bqkv[2 * D : 3 * D][qc]
        # V columns interleaved per head: [wv_h (64 cols) | zeros col]; the
        # ones column (zero weight col + 1.0 bias) carries the row-sum Z.
        wv_i = np.zeros((D, VW), dtype=np.float32)
        bv_i = np.zeros((1, VW), dtype=np.float32)
        for h in range(HPC):
            wv_i[:, 65 * h : 65 * h + 64] = wv[:, 64 * h : 64 * h + 64]
            bv_i[0, 65 * h : 65 * h + 64] = bvv[64 * h : 64 * h + 64]
            bv_i[0, 65 * h + 64] = 1.0
        bqk_cols = np.concatenate([bq, bk]).reshape(4, 128).T  # [128, 4]
        in_maps.append(
            {
                "xT": np.ascontiguousarray(x[b].T.astype(NPB16)),
                "wqkv": np.ascontiguousarray(
                    np.concatenate([wq, wk, wv_i], axis=1).astype(NPB16)
                ),
                "bqk": np.ascontiguousarray(bqk_cols),
                "bv": bv_i.astype(NPB16),
                "wproj": np.ascontiguousarray(
                    Wproj[CD * g : CD * (g + 1), :].astype(NPB16)
                ),
                "onesb": _ONESB,
                "onesr": _ONESR,
                "negi": _NEGI,
                "u128": _U128,
                "zer": _ZER,
            }
        )

    return in_maps


def kernel(x, Wqkv, bqkv, Wproj, bproj):
    global _NC_CACHE, LAST_RESULTS
    x = np.asarray(x, dtype=np.float32)
    Wqkv = np.asarray(Wqkv, dtype=np.float32)
    bqkv = np.asarray(bqkv, dtype=np.float32)
    Wproj = np.asarray(Wproj, dtype=np.float32)
    bproj = np.asarray(bproj, dtype=np.float32)

    if _NC_CACHE is None:
        _NC_CACHE = _build_program()
    nc = _NC_CACHE

    in_maps = _make_in_maps(x, Wqkv, bqkv, Wproj, bproj)
    res = run_bass_kernel_spmd(nc, in_maps, core_ids=list(range(N_CORES)))
    LAST_RESULTS = res

    out = np.empty((B, L, D), dtype=np.float32)
    for b in range(B):
        acc = res.results[4 * b]["yT"].astype(np.float32)
        for g in range(1, 4):
            acc = acc + res.results[4 * b + g]["yT"].astype(np.float32)
        out[b] = acc.T + bproj[None, :]
    return out


# revision 47
# speedup vs baseline: 1.0414x; 1.0054x over previous
"""Causal self-attention (B=2, L=2048, D=1024, H=16) on 8 trn2 NeuronCores.

Sharding: core c = 4*b + g handles batch b and head group g (4 heads).
Per core: QKV projection for its heads' weight columns (tensor-parallel),
flash-style causal attention for its 4 heads, and a partial output
projection over its 256 head-dims (row-parallel).  The host sums the 4
partial projections per batch and adds bproj.

v2 design (single interleaved instruction stream, all-bf16 matmuls):
  - All matmul operands bf16 (FWL weight loads, half DMA); psum stays f32.
  - One software-pipelined stream: QKV chunk s+1 and output-projection
    chunk s-1 matmuls are pulled as PE "filler" between attention steps,
    paced by an emitted-time estimator, so the PE never idles while the
    scalar engine grinds through exps (ACT is the attention-phase
    co-bottleneck at ~1 elem/lane/cycle + ~350cy/call overhead).
  - Score psums are [128, 1024] (2 banks): both heads of a pair per
    k-tile, one merged exp call per step (halves ACT call overhead).
  - Causal diagonal masked ON THE PE: an extra accumulation matmul
    negi^T @ u128 adds -1e30 above the diagonal (no DVE mask adds).
  - AV rides the ones-column trick for Z (col 64 of each head group).
  - Normalize: 1/Z via single-pass custom-DVE reciprocal_approx_fast
    (~51 ULP) into an f32r tile, broadcast across partitions with a
    K=1 ones matmul, psum->sbuf copies, then in-partition multiplies
    reading the AV psum directly (no separate un-normalized copy);
    head1's half goes through a tmp tile + SBUF->SBUF DMA since matmul
    psum bases are restricted to partitions 0/32/64.
  - QKV/V copies out of psum moved to the vector engine (tensor_scalar
    bias-add fused); y copies stay on ACT (bias fused), output bf16.
"""

import sys
import types
from collections import deque

import numpy as np


def _install_ntff_shim():
    """The container's antenv stub lacks axon_hooks; recreate it so
    run_bass_kernel_spmd(trace=True) can reach the NTFF profiler."""
    if "antenv.axon_hooks" in sys.modules:
        return
    try:
        import antenv
        from trn_agent_boot.trn_boot import _ntff_profile_via_ctypes
    except Exception:
        return
    mod = types.ModuleType("antenv.axon_hooks")
    hook = _ntff_profile_via_ctypes("/opt/axon/libaxon_pjrt.so")
    mod.get_axon_ntff_profile_hook = lambda: hook
    mod.set_axon_ntff_profile_hook = lambda h: None
    sys.modules["antenv.axon_hooks"] = mod
    antenv.axon_hooks = mod


_install_ntff_shim()

import ml_dtypes  # noqa: E402

import concourse.bass as bass  # noqa: E402
import concourse.mybir as mybir  # noqa: E402
import concourse.tile as tile  # noqa: E402
from concourse.bass_utils import run_bass_kernel_spmd  # noqa: E402
from concourse.vector_clock import ScopedClock, VectorClock  # noqa: E402

B, L, D, H = 2, 2048, 1024, 16
HD = D // H  # 64
N_CORES = 8
HPC = 4  # heads per core
CD = HPC * HD  # 256 head-dims per core
VW = HPC * (HD + 1)  # 260 interleaved V columns
SCALE = HD**-0.5  # 0.125
F32 = mybir.dt.float32
R32 = mybir.dt.float32r
B16 = mybir.dt.bfloat16
NPB16 = ml_dtypes.bfloat16
NEG = -1.0e30

KT = L // 128  # 16 k-tiles of 128 keys
NS = L // 512  # 4 query chunks of 512
N_DK = D // 128  # 8 feature k-tiles
AV_DELAY = 4  # AV matmuls issue this many merged steps behind their exp
EXPF = mybir.ActivationFunctionType.Exp
IDF = mybir.ActivationFunctionType.Identity
LNF = mybir.ActivationFunctionType.Ln
MULT = mybir.AluOpType.mult


class _TileContext(tile.TileContext):
    """Split exit-drain sem waits to 1 per drain; this walrus build's
    CTRL codegen rejects drains with 2+ sync waits."""

    def _drain_and_barrier(self, tick_clock, wait_clock):
        g = tick_clock.global_clock
        n = len(g)
        procs = [i for i in range(n) if g[i] > 0]
        for p in procs:
            vec = [g[i] if i == p else 0 for i in range(n)]
            d = self.nc.sync.drain()
            wait_clock.add_sem_waits(d.ins, ScopedClock({None: VectorClock(vec)}))
        self.nc.all_engine_barrier()
        popped = self.nc._tile_sem_poison_stack.pop()
        assert popped is self._sem_poison
        self.nc.clear_and_free_semaphores(list(self.sems.allocated().values()))
        self.nc.all_engine_barrier()


def _split_multi_waits(nc):
    """This walrus build's codegen accepts only ONE sync wait per
    instruction; hoist extra waits onto preceding same-engine NOPs."""
    for f in nc.m.functions:
        for blk in f.blocks:
            orig = list(blk.instructions)
            expanded = []
            changed = False
            for ins in orig:
                si = ins.sync_info
                if si is not None and si.on_wait is not None and len(si.on_wait) > 1:
                    changed = True
                    waits = list(si.on_wait)
                    eng = nc.engines[ins.engine]
                    for w in waits[:-1]:
                        nop = eng.nop(nofuse=True).ins
                        nc.cur_bb.bb.instructions.remove(nop)
                        nop.sync_info = mybir.SyncInfo(on_wait=[w], on_update=[])
                        expanded.append(nop)
                    ins.sync_info = mybir.SyncInfo(
                        on_wait=[waits[-1]], on_update=list(si.on_update or [])
                    )
                expanded.append(ins)
            if changed:
                il = blk.instructions
                for ins in list(il):
                    il.remove(ins)
                for ins in expanded:
                    il.append(ins)


def _build_program():
    nc = bass.Bass()
    xT_d = nc.dram_tensor("xT", [D, L], B16, kind="ExternalInput").ap()
    wqkv_d = nc.dram_tensor("wqkv", [D, 2 * CD + VW], B16, kind="ExternalInput").ap()
    bqk_d = nc.dram_tensor("bqk", [128, 4], F32, kind="ExternalInput").ap()
    bv_d = nc.dram_tensor("bv", [1, VW], B16, kind="ExternalInput").ap()
    wproj_d = nc.dram_tensor("wproj", [CD, D], B16, kind="ExternalInput").ap()
    onesb_d = nc.dram_tensor("onesb", [1, 128], B16, kind="ExternalInput").ap()
    onesr_d = nc.dram_tensor("onesr", [1, 64], R32, kind="ExternalInput").ap()
    negi_d = nc.dram_tensor("negi", [128, 128], B16, kind="ExternalInput").ap()
    u128_d = nc.dram_tensor("u128", [128, 128], B16, kind="ExternalInput").ap()
    zer_d = nc.dram_tensor("zer", [64, L], B16, kind="ExternalInput").ap()
    yT_d = nc.dram_tensor("yT", [D, L], B16, kind="ExternalOutput").ap()

    mm = nc.tensor.matmul
    E = {"pe": 0.0, "act": 0.0, "fill": 0.0, "in_fill": False}

    with _TileContext(nc) as tc, tc.tile_pool(name="sb", bufs=1) as sb, tc.tile_pool(
        name="ps", bufs=1, space="PSUM"
    ) as ps, nc.allow_low_precision(reason="bf16 attention kernel"):
        # ---- constants (issued on the ACT HWDGE ring: the sync ring's
        # trigger FIFO (~0.6us each) must stay clear for the critical
        # weight/activation loads) ----
        negi = sb.tile([128, 128], B16, tag="negi", bufs=1)
        nc.scalar.dma_start(out=negi[:], in_=negi_d[:])
        u128 = sb.tile([128, 128], B16, tag="u128", bufs=1)
        nc.scalar.dma_start(out=u128[:], in_=u128_d[:])
        onesb = sb.tile([1, 128], B16, tag="onesb", bufs=1)
        nc.scalar.dma_start(out=onesb[:], in_=onesb_d[:])
        onesr = sb.tile([1, 64], R32, tag="onesr", bufs=1)
        nc.scalar.dma_start(out=onesr[:], in_=onesr_d[:])
        bqk = sb.tile([128, 4], F32, tag="bqk", bufs=1)
        nc.scalar.dma_start(out=bqk[:], in_=bqk_d[:])
        bv = sb.tile([1, VW], B16, tag="bv", bufs=1)
        nc.scalar.dma_start(out=bv[:], in_=bv_d[:])

        # ---- persistent SBUF tensors + DMA schedule ----
        # One big tile per tensor with per-k column views: a single batched
        # DMA per group (one ~0.6us sync-ring trigger instead of 8).
        WQW = 2 * CD + VW  # 772
        wqkv_t = sb.tile([128, N_DK * WQW], B16, tag="wqkv", bufs=1, name="wqkv")
        wqkv = [wqkv_t[:, WQW * k : WQW * (k + 1)] for k in range(N_DK)]
        wqkv_src = wqkv_d.rearrange("(k p) c -> p k c", k=N_DK)
        wqkv_dst = wqkv_t[:].rearrange("p (k c) -> p k c", k=N_DK)
        xT_t = sb.tile([128, N_DK * L], B16, tag="xT", bufs=1, name="xT")
        xTc = [
            [xT_t[:, L * k + 512 * s : L * k + 512 * (s + 1)] for s in range(NS)]
            for k in range(N_DK)
        ]
        xT_src = xT_d.rearrange("(k p) c -> p k c", k=N_DK)
        xT_dst = xT_t[:].rearrange("p (k c) -> p k c", k=N_DK)
        # first-consumption order: w cols 0..255 (m0/m1), x chunk0 for k=0,
        # then the rest of x chunk0, w 256..511, V cols, zero-halves, x 1-3
        wq_groups = [(0, 256), (256, 512), (512, WQW)]
        lo_, hi_ = wq_groups[0]
        nc.sync.dma_start(out=wqkv_dst[:, :, lo_:hi_], in_=wqkv_src[:, :, lo_:hi_])
        nc.sync.dma_start(out=xT_dst[:, 0:1, 0:512], in_=xT_src[:, 0:1, 0:512])
        nc.sync.dma_start(out=xT_dst[:, 1:N_DK, 0:512], in_=xT_src[:, 1:N_DK, 0:512])
        for lo_, hi_ in wq_groups[1:]:
            nc.sync.dma_start(out=wqkv_dst[:, :, lo_:hi_], in_=wqkv_src[:, :, lo_:hi_])
        # K^T zero-padded per head: head h of pair p occupies its 64 rows.
        qT = [sb.tile([128, L], B16, tag=f"qT{p}", bufs=1, name=f"qT{p}") for p in range(2)]
        kz = [
            [
                sb.tile([128, L], B16, tag=f"kz{p}{h}", bufs=1, name=f"kz{p}{h}")
                for h in range(2)
            ]
            for p in range(2)
        ]
        for p in range(2):
            nc.sync.dma_start(out=kz[p][0][64:128, :], in_=zer_d[:])
            nc.sync.dma_start(out=kz[p][1][0:64, :], in_=zer_d[:])
        for s in range(1, NS):
            nc.sync.dma_start(
                out=xT_dst[:, :, 512 * s : 512 * (s + 1)],
                in_=xT_src[:, :, 512 * s : 512 * (s + 1)],
            )
        wproj_t = sb.tile([128, 2 * D], B16, tag="wproj", bufs=1, name="wproj")
        wproj = [wproj_t[:, D * kt : D * (kt + 1)] for kt in range(2)]
        nc.sync.dma_start(
            out=wproj_t[:].rearrange("p (k c) -> p k c", k=2),
            in_=wproj_d.rearrange("(k p) c -> p k c", k=2),
        )
        # V in natural [tok, vcol] layout per k-tile; head hg occupies columns
        # 65*hg..65*hg+64 as [v (64) | 1]; the ones column carries Z.
        vsb = [sb.tile([128, VW], B16, tag=f"v{t}", bufs=1, name=f"v{t}") for t in range(KT)]
        attnT = [
            sb.tile([128, L], B16, tag=f"attnT{k}", bufs=1, name=f"attnT{k}")
            for k in range(2)
        ]

        def mmt(out, lhsT, rhs, cols, **kw):
            mm(out, lhsT, rhs, **kw)
            E["pe"] += cols / 2.4 + 25.0
            if E["in_fill"]:
                E["fill"] += cols / 2.4 + 25.0

        # ---- emitters (generators yield after each PE-significant unit) ----
        def gen_qkv(s):
            cs = slice(512 * s, 512 * (s + 1))
            for m in range(4):
                p_qk = ps.tile([128, 512], F32, tag="mm", bufs=2, name="p_qk")
                for k in range(N_DK):
                    mmt(
                        p_qk[:],
                        wqkv[k][:, 128 * m : 128 * (m + 1)],
                        xTc[k][s][:],
                        cols=512,
                        start=(k == 0),
                        stop=(k == N_DK - 1),
                    )
                    yield
                if m < 2:
                    nc.vector.tensor_scalar_add(qT[m][:, cs], p_qk[:], bqk[:, m : m + 1])
                else:
                    p = m - 2
                    nc.vector.tensor_scalar_add(
                        kz[p][0][0:64, cs], p_qk[0:64, :], bqk[0:64, m : m + 1]
                    )
                    nc.vector.tensor_scalar_add(
                        kz[p][1][64:128, cs], p_qk[64:128, :], bqk[64:128, m : m + 1]
                    )
                yield
            for j in range(4):
                t = 4 * s + j
                p_v = ps.tile([128, VW], F32, tag="mm", bufs=2, name="p_v")
                for k in range(N_DK):
                    mmt(
                        p_v[:],
                        xTc[k][s][:, 128 * j : 128 * (j + 1)],
                        wqkv[k][:, 2 * CD : 2 * CD + VW],
                        cols=VW,
                        start=(k == 0),
                        stop=False,
                    )
                    yield
                mmt(p_v[:], onesb[:], bv[:], cols=VW, start=False, stop=True)
                nc.vector.tensor_copy(vsb[t][:], p_v[:])
                yield

        def gen_proj(s):
            # bproj is added on the host (linear past the partial-sum
            # reduction); the psum->sbuf cast alternates ACT/DVE
            cs = slice(512 * s, 512 * (s + 1))
            for m in range(N_DK):
                p_y = ps.tile([128, 512], F32, tag="mm", bufs=2, name="p_y")
                for kt in range(2):
                    mmt(
                        p_y[:],
                        wproj[kt][:, 128 * m : 128 * (m + 1)],
                        attnT[kt][:, cs],
                        cols=512,
                        start=(kt == 0),
                        stop=(kt == 1),
                    )
                    yield
                # DVE only: y copies on ACT would queue ahead of exps in the
                # strict ACT FIFO and stall the score-psum reuse chain
                y_sb = sb.tile([128, 512], B16, tag="ysb", bufs=6)
                nc.vector.tensor_copy(y_sb[:], p_y[:])
                nc.sync.dma_start(out=yT_d[128 * m : 128 * (m + 1), cs], in_=y_sb[:])
                yield

        class Gen:
            def __init__(self, g):
                self.g = g
                self.done = False

            def step(self):
                if self.done:
                    return False
                try:
                    next(self.g)
                    return True
                except StopIteration:
                    self.done = True
                    return False

        fillers = deque()

        def pull():
            E["in_fill"] = True
            try:
                while fillers:
                    if fillers[0].step():
                        return True
                    fillers.popleft()
                return False
            finally:
                E["in_fill"] = False

        # Pace fillers PROPORTIONALLY to attention-phase ACT progress so all
        # filler work is emitted by the time attention ends (a trailing
        # balance would dump most of proj into a serialized tail).
        FILL_TOTAL = (NS - 1) * (
            4 * N_DK * (512 / 2.4 + 25.0) + 4 * (N_DK + 1) * (VW / 2.4 + 25.0)
        ) + NS * N_DK * 2 * (512 / 2.4 + 25.0)
        ACT_TOTAL = (69632 + 80 * 352) / 1.2 + 32 * (512 + 352) / 1.2

        def balance():
            while E["fill"] < FILL_TOTAL * min(1.0, E["act"] / ACT_TOTAL) and pull():
                pass

        def emit_norm(pair, s, av0, av1):
            # 1/Z = exp(-ln Z) on ACT: ln/exp/identity live in ONE table set
            # (natural_log_exp_and_others) so no table switches.  Ops are
            # batched per engine (both heads' lns, then both exps, ...) to
            # halve the cross-engine serial latency when this chain is
            # exposed at the end of the kernel.
            cs = slice(512 * s, 512 * (s + 1))
            bcs = sb.tile([64, 1024], F32, tag="bcs", bufs=3, name="bcs")
            lnzs, rzs = [], []
            for h, av in ((0, av0), (1, av1)):
                lnz = sb.tile([1, 512], F32, tag="lnz", bufs=6, name="lnz")
                nc.scalar.activation(lnz[:], av[64:65, :], LNF)
                E["act"] += (512 + 352) / 1.2
                lnzs.append(lnz)
            for h in range(2):
                rz = sb.tile([1, 512], R32, tag="rz", bufs=6, name="rz")
                nc.scalar.activation(rz[:], lnzs[h][:], EXPF, scale=-1.0)
                E["act"] += (512 + 352) / 1.2
                rzs.append(rz)
            bc_list = []
            for h in range(2):
                bc_ps = ps.tile([64, 512], F32, tag="mm", bufs=2, name="bc_ps")
                mmt(bc_ps[:], onesr[:], rzs[h][:], cols=512, start=True, stop=True)
                bc_list.append(bc_ps)
            for h in range(2):
                nc.vector.tensor_copy(bcs[:, 512 * h : 512 * (h + 1)], bc_list[h][:])
            nc.vector.tensor_tensor(
                attnT[pair][0:64, cs], av0[0:64, :], bcs[:, 0:512], op=MULT
            )
            tmp = sb.tile([64, 512], B16, tag="ntmp", bufs=3, name="tmp")
            nc.vector.tensor_tensor(tmp[:], av1[0:64, :], bcs[:, 512:1024], op=MULT)
            nc.sync.dma_start(out=attnT[pair][64:128, cs], in_=tmp[:])

        def warmup(n):
            # dummy [128,128] matmuls into a scratch psum: keep the PE busy
            # through front DMA-wait gaps so the HAM clock-gate warms up
            # (~3.4us of sustained activity unlocks 2.4 GHz) instead of
            # idling cold through the whole QKV(0) phase
            wps = ps.tile([128, 128], F32, tag="st", bufs=2, name="warm")
            for _ in range(n):
                mmt(wps[:], negi[:], u128[:], cols=128, start=True, stop=True)

        # ================= QKV chunk 0 =================
        for _ in gen_qkv(0):
            pass
        qkv_gens = {c: Gen(gen_qkv(c)) for c in range(1, NS)}
        fillers.extend(qkv_gens.values())

        # ================= attention (+ interleaved fillers) =================
        pend = []  # (bid, av_mm_args, kw, cols)
        fin_prev = None  # (bid, pair, s, av0, av1)
        for s in range(NS):
            q0 = 512 * s
            # the PE runs in program order: every QKV(s) producer must be
            # EMITTED before attention reads qT/kz/vsb of chunk s, or the
            # score matmul deadlocks waiting on a producer queued behind it
            if s >= 1:
                while not qkv_gens[s].done:
                    pull()
            for pair in range(2):
                bid = 2 * s + pair
                nk = 4 * s + 4
                av0 = ps.tile([128, 512], F32, tag="av", bufs=2, name="av0")
                av1 = ps.tile([128, 512], F32, tag="av", bufs=2, name="av1")
                for k in range(nk):
                    if k == 1 and fin_prev is not None:
                        pbid = fin_prev[0]
                        while pend and pend[0][0] == pbid:
                            _, a, kw, w = pend.pop(0)
                            mmt(*a, cols=w, **kw)
                        emit_norm(*fin_prev[1:])
                        if fin_prev[1] == 1:  # pair 1 normed -> proj(s) ready
                            fillers.append(Gen(gen_proj(fin_prev[2])))
                        fin_prev = None
                    k0 = 128 * k
                    dt_ = k - 4 * s
                    lo = 128 * dt_ if dt_ >= 0 else 0
                    st = ps.tile([128, 1024], F32, tag="st", bufs=2, name="st")
                    for h in range(2):
                        mmt(
                            st[:, 512 * h + lo : 512 * h + 512],
                            kz[pair][h][:, k0 : k0 + 128],
                            qT[pair][:, q0 + lo : q0 + 512],
                            cols=512 - lo,
                            start=True,
                            stop=(dt_ < 0),
                        )
                    if dt_ >= 0:
                        for h in range(2):
                            mmt(
                                st[:, 512 * h + lo : 512 * h + lo + 128],
                                negi[:],
                                u128[:],
                                cols=128,
                                start=False,
                                stop=True,
                                skip_group_check=True,
                            )
                    pt = sb.tile([128, 1024], B16, tag="pt", bufs=AV_DELAY + 5, name="pt")
                    if lo == 0:
                        nc.scalar.activation(pt[:], st[:], EXPF, scale=SCALE)
                        E["act"] += (1024 + 352) / 1.2
                    else:
                        stv = st[:].rearrange("p (h q) -> p h q", h=2)[:, :, lo:512]
                        ptv = pt[:].rearrange("p (h q) -> p h q", h=2)[:, :, lo:512]
                        nc.scalar.activation(ptv, stv, EXPF, scale=SCALE)
                        E["act"] += (2 * (512 - lo) + 352) / 1.2
                    vb = 130 * pair
                    pend.append(
                        (
                            bid,
                            (av0[0:65, lo:512], vsb[k][:, vb : vb + 65], pt[:, lo:512]),
                            dict(start=(k == 0), stop=(k == nk - 1), skip_group_check=True),
                            512 - lo,
                        )
                    )
                    pend.append(
                        (
                            bid,
                            (
                                av1[0:65, lo:512],
                                vsb[k][:, vb + 65 : vb + 130],
                                pt[:, 512 + lo : 1024],
                            ),
                            dict(start=(k == 0), stop=(k == nk - 1), skip_group_check=True),
                            512 - lo,
                        )
                    )
                    while len(pend) > 2 * AV_DELAY:
                        _, a, kw, w = pend.pop(0)
                        mmt(*a, cols=w, **kw)
                    balance()
                fin_prev = (bid, pair, s, av0, av1)
        while pend:
            _, a, kw, w = pend.pop(0)
            mmt(*a, cols=w, **kw)
        emit_norm(*fin_prev[1:])
        fillers.append(Gen(gen_proj(NS - 1)))
        while pull():
            pass
    _split_multi_waits(nc)
    return nc


_NC_CACHE = None
LAST_RESULTS = None

_ONESB = np.ones((1, 128), dtype=NPB16)
_ONESR = np.ones((1, 64), dtype=np.float32)
_I, _J = np.meshgrid(np.arange(128), np.arange(128), indexing="ij")
_NEGI = (np.where(_I == _J, NEG, 0.0)).astype(NPB16)
_U128 = (np.where(_I > _J, 1.0, 0.0)).astype(NPB16)
_ZER = np.zeros((64, L), dtype=NPB16)


def _make_in_maps(x, Wqkv, bqkv, Wproj, bproj):
    in_maps = []
    for c in range(N_CORES):
        b, g = divmod(c, 4)
        qc = slice(CD * g, CD * (g + 1))
        wq = Wqkv[:, qc]
        wk = Wqkv[:, D : 2 * D][:, qc]
        wv = Wqkv[:, 2 * D : 3 * D][:, qc]
        bq = bqkv[qc]
        bk = bqkv[D : 2 * D][qc]
        bvv = bqkv[2 * D : 3 * D][qc]
        # V columns interleaved per head: [wv_h (64 cols) | zeros col]; the
        # ones column (zero weight col + 1.0 bias) carries the row-sum Z.
        wv_i = np.zeros((D, VW), dtype=np.float32)
        bv_i = np.zeros((1, VW), dtype=np.float32)
        for h in range(HPC):
            wv_i[:, 65 * h : 65 * h + 64] = wv[:, 64 * h : 64 * h + 64]
            bv_i[0, 65 * h : 65 * h + 64] = bvv[64 * h : 64 * h + 64]
            bv_i[0, 65 * h + 64] = 1.0
        bqk_cols = np.concatenate([bq, bk]).reshape(4, 128).T  # [128, 4]
        in_maps.append(
            {
                "xT": np.ascontiguousarray(x[b].T.astype(NPB16)),
                "wqkv": np.ascontiguousarray(
                    np.concatenate([wq, wk, wv_i], axis=1).astype(NPB16)
                ),
                "bqk": np.ascontiguousarray(bqk_cols),
                "bv": bv_i.astype(NPB16),
                "wproj": np.ascontiguousarray(
                    Wproj[CD * g : CD * (g + 1), :].astype(NPB16)
                ),
                "onesb": _ONESB,
                "onesr": _ONESR,
                "negi": _NEGI,
                "u128": _U128,
                "zer": _ZER,
            }
        )

    return in_maps


def kernel(x, Wqkv, bqkv, Wproj, bproj):
    global _NC_CACHE, LAST_RESULTS
    x = np.asarray(x, dtype=np.float32)
    Wqkv = np.asarray(Wqkv, dtype=np.float32)
    bqkv = np.asarray(bqkv, dtype=np.float32)
    Wproj = np.asarray(Wproj, dtype=np.float32)
    bproj = np.asarray(bproj, dtype=np.float32)

    if _NC_CACHE is None:
        _NC_CACHE = _build_program()
    nc = _NC_CACHE

    in_maps = _make_in_maps(x, Wqkv, bqkv, Wproj, bproj)
    res = run_bass_kernel_spmd(nc, in_maps, core_ids=list(range(N_CORES)))
    LAST_RESULTS = res

    out = np.empty((B, L, D), dtype=np.float32)
    for b in range(B):
        acc = res.results[4 * b]["yT"].astype(np.float32)
        for g in range(1, 4):
            acc = acc + res.results[4 * b + g]["yT"].astype(np.float32)
        out[b] = acc.T + bproj[None, :]
    return out
